# revision 5
# baseline (speedup 1.0000x reference)
"""Trainium2 Bass kernel for nn_AlignmentMatrix.

score[b,i,j] = [ctx_i ; asp_j ; ctx_i*asp_j] @ w_u
            = sum_d ctx[b,i,d]*w3[d]*asp[b,j,d] + ctx[b]@w1 + asp[b]@w2

Reformulated per batch as:
    out[b] = ctx[b] @ R[b] + asp_term[b][None, :]
with
    R[b][d, j]  = w3[d]*asp[b,j,d] + w1[d]      (folds the ctx@w1 bias in,
                                                 since sum_d ctx[i,d]*w1[d])
    asp_term[b] = asp[b] @ w2

Data-parallel across the batch dim on 8 NeuronCores (8 batches/core).
On each core, per batch:
  - asp tiles are loaded naturally, transposed on the PE (identity matmul)
    into [D, La] layout, then R is formed with one fused DVE
    tensor_scalar (mul w3, add w1) per K-chunk.
  - asp_term is computed replicated across all 128 partitions directly,
    using a stationary operand w2rep[d, m] = w2[d] (same value in every
    column), so no cross-partition broadcast is ever needed.
  - ctx tiles are loaded naturally ([128, 400] contiguous 200KB DMAs),
    transposed on the PE, and used as the stationary operand of the main
    matmul (float32r: full-rate fp32 path, N=512) accumulating over the
    4 K-chunks of D=400 into one PSUM bank.
  - DVE moves PSUM->SBUF adding asp_term_rep; output DMA'd back (256KB
    contiguous per tile).
"""

import numpy as np

import concourse.bass as bass
from concourse import bacc
import concourse.mybir as mybir
import concourse.tile as tile
from concourse.bass_utils import run_bass_kernel_spmd
from concourse.masks import make_identity

F32 = mybir.dt.float32
F32R = mybir.dt.float32r

B, LC, LA, D = 64, 2048, 512, 400
N_CORES = 8
PB = B // N_CORES  # batches per core
# K chunks of the contraction dim D = 400 = 3*128 + 16
KCHUNKS = [(0, 128), (128, 128), (256, 128), (384, 16)]
P = 128


def build_kernel(pb: int = PB, lc: int = LC) -> bass.Bass:
    nc = bacc.Bacc(
        "TRN2",
        target_bir_lowering=False,
        debug=False,
        num_devices=N_CORES,
    )
    ctx_d = nc.dram_tensor("ctx", [pb, lc, D], F32, kind="ExternalInput").ap()
    asp_d = nc.dram_tensor("asp", [pb, LA, D], F32, kind="ExternalInput").ap()
    wu_d = nc.dram_tensor("w_u", [3 * D, 1], F32, kind="ExternalInput").ap()
    out_d = nc.dram_tensor("out", [pb, lc, LA], F32, kind="ExternalOutput").ap()

    with tile.TileContext(nc) as tc:
        _kernel_body(tc, out_d, ctx_d, asp_d, wu_d, pb, lc)
    nc.compile()
    return nc


def _kernel_body(tc, out_d, ctx_d, asp_d, wu_d, pb, lc):
    nc = tc.nc
    n_lc_tiles = lc // P

    const = tc.alloc_tile_pool(name="const", bufs=1)
    ctx_pool = tc.alloc_tile_pool(name="ctxN", bufs=4)
    ctxT_pool = tc.alloc_tile_pool(name="ctxT", bufs=4)
    asp_pool = tc.alloc_tile_pool(name="aspN", bufs=8)
    aspT_pool = tc.alloc_tile_pool(name="aspT", bufs=2)
    rhsp_pool = tc.alloc_tile_pool(name="rhsp", bufs=2)
    atrep_pool = tc.alloc_tile_pool(name="atrep", bufs=2)
    out_pool = tc.alloc_tile_pool(name="outT", bufs=4)
    psum_t = tc.alloc_tile_pool(name="psumT", bufs=3, space="PSUM")
    psum_o = tc.alloc_tile_pool(name="psumO", bufs=3, space="PSUM")
    psum_a = tc.alloc_tile_pool(name="psumA", bufs=2, space="PSUM")

    ident = const.tile([P, P], F32, tag="ident", name="ident")
    make_identity(nc, ident)

    # Per K-chunk weight columns: w1 (add bias), w3 (mul scale) as [kc, 1]
    # per-partition scalars, and w2rep [kc, 128] = w2 replicated along
    # columns (stationary operand that produces asp_term broadcast over
    # all output partitions).
    w1c, w3c, w2rep = [], [], []
    for c, (d0, kc) in enumerate(KCHUNKS):
        w1t = const.tile([kc, 1], F32, tag=f"w1_{c}", name=f"w1_{c}")
        nc.sync.dma_start(out=w1t, in_=wu_d[d0 : d0 + kc, :])
        w3t = const.tile([kc, 1], F32, tag=f"w3_{c}", name=f"w3_{c}")
        nc.sync.dma_start(out=w3t, in_=wu_d[2 * D + d0 : 2 * D + d0 + kc, :])
        w2t = const.tile([kc, 1], F32, tag=f"w2_{c}", name=f"w2_{c}")
        nc.sync.dma_start(out=w2t, in_=wu_d[D + d0 : D + d0 + kc, :])
        w2r = const.tile([kc, P], F32R, tag=f"w2rep_{c}", name=f"w2rep_{c}")
        nc.vector.tensor_copy(w2r, w2t.to_broadcast((kc, P)))
        w1c.append(w1t)
        w3c.append(w3t)
        w2rep.append(w2r)

    for b in range(pb):
        # ---------------- asp prep for batch b ----------------
        asp_tiles = []
        for t in range(LA // P):
            aspN = asp_pool.tile([P, D], F32, tag="aspN", name=f"aspN_{b}_{t}")
            nc.sync.dma_start(out=aspN, in_=asp_d[b, t * P : (t + 1) * P, :])
            asp_tiles.append(aspN)

        rhsp = []
        pat = psum_a.tile([P, LA], F32, tag="pat", name=f"pat_{b}")
        for c, (d0, kc) in enumerate(KCHUNKS):
            pT = psum_t.tile([P, 4 * P], F32, tag="pT", name=f"pTa_{b}_{c}")
            for t in range(LA // P):
                nc.tensor.transpose(
                    pT[0:kc, t * P : (t + 1) * P],
                    asp_tiles[t][:, d0 : d0 + kc],
                    ident,
                )
            a_raw = aspT_pool.tile([kc, LA], F32R, tag=f"aspT_{c}", name=f"aspT_{b}_{c}")
            nc.scalar.copy(a_raw, pT[0:kc, :])
            # R chunk = w3 * aspT + w1 (per-partition scalars)
            rp = rhsp_pool.tile([kc, LA], F32R, tag=f"rhsp_{c}", name=f"rhsp_{b}_{c}")
            nc.vector.tensor_scalar(
                rp,
                a_raw,
                w3c[c],
                w1c[c],
                op0=mybir.AluOpType.mult,
                op1=mybir.AluOpType.add,
            )
            rhsp.append(rp)
            # asp_term replicated: pat[m, j] += sum_d w2[d] * aspT[d, j]
            nc.tensor.matmul(
                pat,
                w2rep[c],
                a_raw,
                start=(c == 0),
                stop=(c == len(KCHUNKS) - 1),
            )
        at_rep = atrep_pool.tile([P, LA], F32, tag="atrep", name=f"atrep_{b}")
        nc.scalar.copy(at_rep, pat)

        # ---------------- ctx tiles for batch b ----------------
        for t in range(n_lc_tiles):
            ctxN = ctx_pool.tile([P, D], F32, tag="ctxN", name=f"ctxN_{b}_{t}")
            nc.sync.dma_start(out=ctxN, in_=ctx_d[b, t * P : (t + 1) * P, :])
            pT = psum_t.tile([P, 4 * P], F32, tag="pT", name=f"pTc_{b}_{t}")
            for c, (d0, kc) in enumerate(KCHUNKS):
                nc.tensor.transpose(
                    pT[0:kc, c * P : (c + 1) * P],
                    ctxN[:, d0 : d0 + kc],
                    ident,
                )
            ctxT = ctxT_pool.tile([P, 4 * P], F32R, tag="ctxT", name=f"ctxT_{b}_{t}")
            # chunks 0-2 occupy full 128 partitions; chunk 3 only 16
            nc.scalar.copy(ctxT[:, 0 : 3 * P], pT[:, 0 : 3 * P])
            nc.scalar.copy(ctxT[0:16, 3 * P : 4 * P], pT[0:16, 3 * P : 4 * P])

            pO = psum_o.tile([P, LA], F32, tag="pO", name=f"pO_{b}_{t}")
            for c, (d0, kc) in enumerate(KCHUNKS):
                nc.tensor.matmul(
                    pO,
                    ctxT[0:kc, c * P : (c + 1) * P],
                    rhsp[c],
                    start=(c == 0),
                    stop=(c == len(KCHUNKS) - 1),
                )
            ot = out_pool.tile([P, LA], F32, tag="ot", name=f"ot_{b}_{t}")
            nc.vector.tensor_add(ot, pO, at_rep)
            nc.sync.dma_start(out=out_d[b, t * P : (t + 1) * P, :], in_=ot)

    for p in reversed((const, ctx_pool, ctxT_pool, asp_pool, aspT_pool, rhsp_pool,
                       atrep_pool, out_pool, psum_t, psum_o, psum_a)):
        p.release()


def kernel(batch_size=None, ctx=None, asp=None, w_u=None, **_unused):
    ctx = np.ascontiguousarray(np.asarray(ctx, dtype=np.float32))
    asp = np.ascontiguousarray(np.asarray(asp, dtype=np.float32))
    w_u = np.ascontiguousarray(np.asarray(w_u, dtype=np.float32))

    nc = build_kernel()
    in_maps = [
        {
            "ctx": ctx[i * PB : (i + 1) * PB],
            "asp": asp[i * PB : (i + 1) * PB],
            "w_u": w_u,
        }
        for i in range(N_CORES)
    ]
    res = run_bass_kernel_spmd(
        nc, in_maps, core_ids=list(range(N_CORES)), **_RUN_KWARGS
    )
    _LAST_RESULTS.clear()
    _LAST_RESULTS.append(res)
    out = np.concatenate([res.results[i]["out"] for i in range(N_CORES)], axis=0)
    return out


# test-harness hooks: extra kwargs for run_bass_kernel_spmd (e.g. trace=True)
# and the last BassKernelResults for profiling. Unused in grading.
_RUN_KWARGS: dict = {}
_LAST_RESULTS: list = []


# revision 6
# speedup vs baseline: 1.5300x; 1.5300x over previous
"""Trainium2 Bass kernel for nn_AlignmentMatrix.

score[b,i,j] = [ctx_i ; asp_j ; ctx_i*asp_j] @ w_u
            = sum_d ctx[b,i,d]*w3[d]*asp[b,j,d] + ctx[b]@w1 + asp[b]@w2

Reformulated per batch as:
    out[b] = ctx[b] @ R[b] + asp_term[b][None, :]
with
    R[b][d, j]  = w3[d]*asp[b,j,d] + w1[d]      (folds the ctx@w1 bias in,
                                                 since sum_d ctx[i,d]*w1[d])
    asp_term[b] = asp[b] @ w2

Data-parallel across the batch dim on 8 NeuronCores (8 batches/core).
On each core, per batch:
  - ctx/asp are DMA'd in with an inline f32->bf16 cast (SWDGE), 4 row-tiles
    per transfer (800KB HBM reads) for DMA efficiency.
  - natural-layout bf16 tiles are transposed on the PE (single-pass bf16
    transpose mode) into [D, *] layout; ScalarE copies PSUM->SBUF.
  - R is formed with one fused DVE tensor_scalar (mul w3, add w1) per
    K-chunk; asp_term is produced already replicated across partitions by
    a w2-replicated stationary matmul.
  - main matmul in bf16 (fast weight load + 2 col/cycle streaming),
    fp32 PSUM accumulation over the 4 K-chunks of D=400.
  - DVE adds asp_term during the PSUM->SBUF move (fp32); output stores
    are 1MB contiguous DMAs (4 row-tiles each).
"""

import numpy as np

import concourse.bass as bass
from concourse import bacc
import concourse.mybir as mybir
import concourse.tile as tile
from concourse.bass_utils import run_bass_kernel_spmd
from concourse.masks import make_identity

F32 = mybir.dt.float32
BF16 = mybir.dt.bfloat16

B, LC, LA, D = 64, 2048, 512, 400
N_CORES = 8
PB = B // N_CORES  # batches per core
# K chunks of the contraction dim D = 400 = 3*128 + 16
KCHUNKS = [(0, 128), (128, 128), (256, 128), (384, 16)]
P = 128
TT = 4  # row-tiles per DMA transfer


def build_kernel(pb: int = PB, lc: int = LC) -> bass.Bass:
    nc = bacc.Bacc(
        "TRN2",
        target_bir_lowering=False,
        debug=False,
        num_devices=N_CORES,
    )
    ctx_d = nc.dram_tensor("ctx", [pb, lc, D], F32, kind="ExternalInput").ap()
    asp_d = nc.dram_tensor("asp", [pb, LA, D], F32, kind="ExternalInput").ap()
    wu_d = nc.dram_tensor("w_u", [3 * D, 1], F32, kind="ExternalInput").ap()
    out_d = nc.dram_tensor("out", [pb, lc, LA], F32, kind="ExternalOutput").ap()

    with tile.TileContext(nc) as tc:
        _kernel_body(tc, out_d, ctx_d, asp_d, wu_d, pb, lc)
    nc.compile()
    return nc


def _kernel_body(tc, out_d, ctx_d, asp_d, wu_d, pb, lc):
    nc = tc.nc
    n_lc_tiles = lc // P
    n_groups = n_lc_tiles // TT  # ctx tile-groups per batch

    const = tc.alloc_tile_pool(name="const", bufs=1)
    ctx_pool = tc.alloc_tile_pool(name="ctxN", bufs=3)
    ctxT_pool = tc.alloc_tile_pool(name="ctxT", bufs=4)
    asp_pool = tc.alloc_tile_pool(name="aspN", bufs=2)
    aspT_pool = tc.alloc_tile_pool(name="aspT", bufs=2)
    rhsp_pool = tc.alloc_tile_pool(name="rhsp", bufs=2)
    atrep_pool = tc.alloc_tile_pool(name="atrep", bufs=2)
    out_pool = tc.alloc_tile_pool(name="outT", bufs=3)
    psum_t = tc.alloc_tile_pool(name="psumT", bufs=3, space="PSUM")
    psum_o = tc.alloc_tile_pool(name="psumO", bufs=3, space="PSUM")
    psum_a = tc.alloc_tile_pool(name="psumA", bufs=2, space="PSUM")

    ident = const.tile([P, P], BF16, tag="ident", name="ident")
    make_identity(nc, ident)

    # Per K-chunk weight columns: w1 (add bias), w3 (mul scale) as [kc, 1]
    # fp32 per-partition scalars, and w2rep [kc, 128] bf16 = w2 replicated
    # along columns (stationary operand producing asp_term replicated over
    # all output partitions).
    w1c, w3c, w2rep = [], [], []
    for c, (d0, kc) in enumerate(KCHUNKS):
        w1t = const.tile([kc, 1], F32, tag=f"w1_{c}", name=f"w1_{c}")
        nc.sync.dma_start(out=w1t, in_=wu_d[d0 : d0 + kc, :])
        w3t = const.tile([kc, 1], F32, tag=f"w3_{c}", name=f"w3_{c}")
        nc.sync.dma_start(out=w3t, in_=wu_d[2 * D + d0 : 2 * D + d0 + kc, :])
        w2t = const.tile([kc, 1], F32, tag=f"w2_{c}", name=f"w2_{c}")
        nc.sync.dma_start(out=w2t, in_=wu_d[D + d0 : D + d0 + kc, :])
        w2r = const.tile([kc, P], BF16, tag=f"w2rep_{c}", name=f"w2rep_{c}")
        nc.vector.tensor_copy(w2r, w2t.to_broadcast((kc, P)))
        w1c.append(w1t)
        w3c.append(w3t)
        w2rep.append(w2r)

    for b in range(pb):
        # ---------------- asp prep for batch b ----------------
        # One 800KB DMA, f32->bf16 cast inline: [512, 400] -> [128, 4*400]
        aspN = asp_pool.tile([P, TT * D], BF16, tag="aspN", name=f"aspN_{b}")
        nc.gpsimd.dma_start(
            out=aspN.rearrange("p (tt d) -> p tt d", tt=TT),
            in_=asp_d[b].rearrange("(tt p) d -> p tt d", p=P),
        )

        rhsp = []
        pat = psum_a.tile([P, LA], F32, tag="pat", name=f"pat_{b}")
        for c, (d0, kc) in enumerate(KCHUNKS):
            pT = psum_t.tile([P, TT * P], BF16, tag="pT", name=f"pTa_{b}_{c}")
            for t in range(LA // P):
                nc.tensor.transpose(
                    pT[0:kc, t * P : (t + 1) * P],
                    aspN[:, t * D + d0 : t * D + d0 + kc],
                    ident,
                )
            a_raw = aspT_pool.tile([kc, LA], BF16, tag=f"aspT_{c}", name=f"aspT_{b}_{c}")
            nc.scalar.copy(a_raw, pT[0:kc, :])
            # R chunk = w3 * aspT + w1 (per-partition fp32 scalars), bf16 out
            rp = rhsp_pool.tile([kc, LA], BF16, tag=f"rhsp_{c}", name=f"rhsp_{b}_{c}")
            nc.vector.tensor_scalar(
                rp,
                a_raw,
                w3c[c],
                w1c[c],
                op0=mybir.AluOpType.mult,
                op1=mybir.AluOpType.add,
            )
            rhsp.append(rp)
            # asp_term replicated: pat[m, j] += sum_d w2[d] * aspT[d, j]
            nc.tensor.matmul(
                pat,
                w2rep[c],
                a_raw,
                start=(c == 0),
                stop=(c == len(KCHUNKS) - 1),
            )
        at_rep = atrep_pool.tile([P, LA], F32, tag="atrep", name=f"atrep_{b}")
        nc.scalar.copy(at_rep, pat)

        # ---------------- ctx tiles for batch b ----------------
        for g in range(n_groups):
            # 800KB DMA with inline cast: 4 row-tiles [512, 400] -> [128, 1600]
            ctxN = ctx_pool.tile([P, TT * D], BF16, tag="ctxN", name=f"ctxN_{b}_{g}")
            r0 = g * TT * P
            nc.gpsimd.dma_start(
                out=ctxN.rearrange("p (tt d) -> p tt d", tt=TT),
                in_=ctx_d[b, r0 : r0 + TT * P, :].rearrange(
                    "(tt p) d -> p tt d", p=P
                ),
            )
            ot = out_pool.tile([P, TT * LA], F32, tag="ot", name=f"ot_{b}_{g}")
            for tt in range(TT):
                pT = psum_t.tile([P, TT * P], BF16, tag="pT", name=f"pTc_{b}_{g}_{tt}")
                for c, (d0, kc) in enumerate(KCHUNKS):
                    nc.tensor.transpose(
                        pT[0:kc, c * P : (c + 1) * P],
                        ctxN[:, tt * D + d0 : tt * D + d0 + kc],
                        ident,
                    )
                ctxT = ctxT_pool.tile([P, TT * P], BF16, tag="ctxT", name=f"ctxT_{b}_{g}_{tt}")
                # chunks 0-2 occupy full 128 partitions; chunk 3 only 16
                nc.scalar.copy(ctxT[:, 0 : 3 * P], pT[:, 0 : 3 * P])
                nc.scalar.copy(ctxT[0:16, 3 * P : 4 * P], pT[0:16, 3 * P : 4 * P])

                pO = psum_o.tile([P, LA], F32, tag="pO", name=f"pO_{b}_{g}_{tt}")
                for c, (d0, kc) in enumerate(KCHUNKS):
                    nc.tensor.matmul(
                        pO,
                        ctxT[0:kc, c * P : (c + 1) * P],
                        rhsp[c],
                        start=(c == 0),
                        stop=(c == len(KCHUNKS) - 1),
                    )
                nc.vector.tensor_add(ot[:, tt * LA : (tt + 1) * LA], pO, at_rep)
            # 1MB contiguous store: [128, 4*512] -> [512, 512] rows
            nc.sync.dma_start(
                out=out_d[b, r0 : r0 + TT * P, :].rearrange(
                    "(tt p) j -> p tt j", p=P
                ),
                in_=ot.rearrange("p (tt j) -> p tt j", tt=TT),
            )

    for p in reversed((const, ctx_pool, ctxT_pool, asp_pool, aspT_pool, rhsp_pool,
                       atrep_pool, out_pool, psum_t, psum_o, psum_a)):
        p.release()


def kernel(batch_size=None, ctx=None, asp=None, w_u=None, **_unused):
    ctx = np.ascontiguousarray(np.asarray(ctx, dtype=np.float32))
    asp = np.ascontiguousarray(np.asarray(asp, dtype=np.float32))
    w_u = np.ascontiguousarray(np.asarray(w_u, dtype=np.float32))

    nc = build_kernel()
    in_maps = [
        {
            "ctx": ctx[i * PB : (i + 1) * PB],
            "asp": asp[i * PB : (i + 1) * PB],
            "w_u": w_u,
        }
        for i in range(N_CORES)
    ]
    res = run_bass_kernel_spmd(
        nc, in_maps, core_ids=list(range(N_CORES)), **_RUN_KWARGS
    )
    _LAST_RESULTS.clear()
    _LAST_RESULTS.append(res)
    out = np.concatenate([res.results[i]["out"] for i in range(N_CORES)], axis=0)
    return out


# test-harness hooks: extra kwargs for run_bass_kernel_spmd (e.g. trace=True)
# and the last BassKernelResults for profiling. Unused in grading.
_RUN_KWARGS: dict = {}
_LAST_RESULTS: list = []


# revision 7
# speedup vs baseline: 1.5714x; 1.0270x over previous
"""Trainium2 Bass kernel for nn_AlignmentMatrix.

score[b,i,j] = [ctx_i ; asp_j ; ctx_i*asp_j] @ w_u
            = sum_d ctx[b,i,d]*w3[d]*asp[b,j,d] + ctx[b]@w1 + asp[b]@w2

Reformulated per batch as:
    out[b] = ctx[b] @ R[b] + asp_term[b][None, :]
with
    R[b][d, j]  = w3[d]*asp[b,j,d] + w1[d]      (folds the ctx@w1 bias in,
                                                 since sum_d ctx[i,d]*w1[d])
    asp_term[b] = asp[b] @ w2

Data-parallel across the batch dim on 8 NeuronCores (8 batches/core).
On each core, per batch:
  - ctx/asp are DMA'd in with an inline f32->bf16 cast (SWDGE), 4 row-tiles
    per transfer (800KB HBM reads) for DMA efficiency.
  - natural-layout bf16 tiles are transposed on the PE (single-pass bf16
    transpose mode) into [D, *] layout; ScalarE copies PSUM->SBUF.
  - R is formed with one fused DVE tensor_scalar (mul w3, add w1) per
    K-chunk; asp_term is produced already replicated across partitions by
    a w2-replicated stationary matmul.
  - main matmul in bf16 (fast weight load + 2 col/cycle streaming),
    fp32 PSUM accumulation over the 4 K-chunks of D=400.
  - DVE adds asp_term during the PSUM->SBUF move (fp32); output stores
    are 1MB contiguous DMAs (4 row-tiles each).
"""

import numpy as np

import concourse.bass as bass
from concourse import bacc
import concourse.mybir as mybir
import concourse.tile as tile
from concourse.bass_utils import run_bass_kernel_spmd
from concourse.masks import make_identity

F32 = mybir.dt.float32
F16 = mybir.dt.float16

B, LC, LA, D = 64, 2048, 512, 400
N_CORES = 8
PB = B // N_CORES  # batches per core
# K chunks of the contraction dim D = 400 = 3*128 + 16
KCHUNKS = [(0, 128), (128, 128), (256, 128), (384, 16)]
P = 128
TT = 4  # row-tiles per DMA transfer


def build_kernel(pb: int = PB, lc: int = LC) -> bass.Bass:
    nc = bacc.Bacc(
        "TRN2",
        target_bir_lowering=False,
        debug=False,
        num_devices=N_CORES,
    )
    ctx_d = nc.dram_tensor("ctx", [pb, lc, D], F32, kind="ExternalInput").ap()
    asp_d = nc.dram_tensor("asp", [pb, LA, D], F32, kind="ExternalInput").ap()
    wu_d = nc.dram_tensor("w_u", [3 * D, 1], F32, kind="ExternalInput").ap()
    out_d = nc.dram_tensor("out", [pb, lc, LA], F16, kind="ExternalOutput").ap()

    with tile.TileContext(nc) as tc:
        _kernel_body(tc, out_d, ctx_d, asp_d, wu_d, pb, lc)
    nc.compile()
    return nc


def _kernel_body(tc, out_d, ctx_d, asp_d, wu_d, pb, lc):
    nc = tc.nc
    n_lc_tiles = lc // P
    n_groups = n_lc_tiles // TT  # ctx tile-groups per batch

    const = tc.alloc_tile_pool(name="const", bufs=1)
    ctx_pool = tc.alloc_tile_pool(name="ctxN", bufs=3)
    ctxT_pool = tc.alloc_tile_pool(name="ctxT", bufs=4)
    asp_pool = tc.alloc_tile_pool(name="aspN", bufs=2)
    aspT_pool = tc.alloc_tile_pool(name="aspT", bufs=2)
    rhsp_pool = tc.alloc_tile_pool(name="rhsp", bufs=2)
    atrep_pool = tc.alloc_tile_pool(name="atrep", bufs=2)
    out_pool = tc.alloc_tile_pool(name="outT", bufs=3)
    psum_t = tc.alloc_tile_pool(name="psumT", bufs=3, space="PSUM")
    psum_o = tc.alloc_tile_pool(name="psumO", bufs=3, space="PSUM")
    psum_a = tc.alloc_tile_pool(name="psumA", bufs=2, space="PSUM")

    ident = const.tile([P, P], F16, tag="ident", name="ident")
    make_identity(nc, ident)

    # Per K-chunk weight columns: w1 (add bias), w3 (mul scale) as [kc, 1]
    # fp32 per-partition scalars, and w2rep [kc, 128] bf16 = w2 replicated
    # along columns (stationary operand producing asp_term replicated over
    # all output partitions).
    w1c, w3c, w2rep = [], [], []
    for c, (d0, kc) in enumerate(KCHUNKS):
        w1t = const.tile([kc, 1], F32, tag=f"w1_{c}", name=f"w1_{c}")
        nc.sync.dma_start(out=w1t, in_=wu_d[d0 : d0 + kc, :])
        w3t = const.tile([kc, 1], F32, tag=f"w3_{c}", name=f"w3_{c}")
        nc.sync.dma_start(out=w3t, in_=wu_d[2 * D + d0 : 2 * D + d0 + kc, :])
        w2t = const.tile([kc, 1], F32, tag=f"w2_{c}", name=f"w2_{c}")
        nc.sync.dma_start(out=w2t, in_=wu_d[D + d0 : D + d0 + kc, :])
        w2r = const.tile([kc, P], F16, tag=f"w2rep_{c}", name=f"w2rep_{c}")
        nc.vector.tensor_copy(w2r, w2t.to_broadcast((kc, P)))
        w1c.append(w1t)
        w3c.append(w3t)
        w2rep.append(w2r)

    for b in range(pb):
        # ---------------- asp prep for batch b ----------------
        # One 800KB DMA, f32->bf16 cast inline: [512, 400] -> [128, 4*400]
        aspN = asp_pool.tile([P, TT * D], F16, tag="aspN", name=f"aspN_{b}")
        nc.gpsimd.dma_start(
            out=aspN.rearrange("p (tt d) -> p tt d", tt=TT),
            in_=asp_d[b].rearrange("(tt p) d -> p tt d", p=P),
        )

        rhsp = []
        pat = psum_a.tile([P, LA], F32, tag="pat", name=f"pat_{b}")
        for c, (d0, kc) in enumerate(KCHUNKS):
            pT = psum_t.tile([P, TT * P], F16, tag="pT", name=f"pTa_{b}_{c}")
            for t in range(LA // P):
                nc.tensor.transpose(
                    pT[0:kc, t * P : (t + 1) * P],
                    aspN[:, t * D + d0 : t * D + d0 + kc],
                    ident,
                )
            a_raw = aspT_pool.tile([kc, LA], F16, tag=f"aspT_{c}", name=f"aspT_{b}_{c}")
            nc.scalar.copy(a_raw, pT[0:kc, :])
            # R chunk = w3 * aspT + w1 (per-partition fp32 scalars), bf16 out
            rp = rhsp_pool.tile([kc, LA], F16, tag=f"rhsp_{c}", name=f"rhsp_{b}_{c}")
            nc.vector.tensor_scalar(
                rp,
                a_raw,
                w3c[c],
                w1c[c],
                op0=mybir.AluOpType.mult,
                op1=mybir.AluOpType.add,
            )
            rhsp.append(rp)
            # asp_term replicated: pat[m, j] += sum_d w2[d] * aspT[d, j]
            nc.tensor.matmul(
                pat,
                w2rep[c],
                a_raw,
                start=(c == 0),
                stop=(c == len(KCHUNKS) - 1),
            )
        at_rep = atrep_pool.tile([P, LA], F32, tag="atrep", name=f"atrep_{b}")
        nc.scalar.copy(at_rep, pat)

        # ---------------- ctx tiles for batch b ----------------
        for g in range(n_groups):
            # 800KB DMA with inline cast: 4 row-tiles [512, 400] -> [128, 1600]
            ctxN = ctx_pool.tile([P, TT * D], F16, tag="ctxN", name=f"ctxN_{b}_{g}")
            r0 = g * TT * P
            nc.gpsimd.dma_start(
                out=ctxN.rearrange("p (tt d) -> p tt d", tt=TT),
                in_=ctx_d[b, r0 : r0 + TT * P, :].rearrange(
                    "(tt p) d -> p tt d", p=P
                ),
            )
            ot = out_pool.tile([P, TT * LA], F16, tag="ot", name=f"ot_{b}_{g}")
            for tt in range(TT):
                pT = psum_t.tile([P, TT * P], F16, tag="pT", name=f"pTc_{b}_{g}_{tt}")
                for c, (d0, kc) in enumerate(KCHUNKS):
                    nc.tensor.transpose(
                        pT[0:kc, c * P : (c + 1) * P],
                        ctxN[:, tt * D + d0 : tt * D + d0 + kc],
                        ident,
                    )
                ctxT = ctxT_pool.tile([P, TT * P], F16, tag="ctxT", name=f"ctxT_{b}_{g}_{tt}")
                # chunks 0-2 occupy full 128 partitions; chunk 3 only 16
                nc.scalar.copy(ctxT[:, 0 : 3 * P], pT[:, 0 : 3 * P])
                nc.scalar.copy(ctxT[0:16, 3 * P : 4 * P], pT[0:16, 3 * P : 4 * P])

                pO = psum_o.tile([P, LA], F32, tag="pO", name=f"pO_{b}_{g}_{tt}")
                for c, (d0, kc) in enumerate(KCHUNKS):
                    nc.tensor.matmul(
                        pO,
                        ctxT[0:kc, c * P : (c + 1) * P],
                        rhsp[c],
                        start=(c == 0),
                        stop=(c == len(KCHUNKS) - 1),
                    )
                nc.vector.tensor_add(ot[:, tt * LA : (tt + 1) * LA], pO, at_rep)
            # 1MB contiguous store: [128, 4*512] -> [512, 512] rows
            nc.sync.dma_start(
                out=out_d[b, r0 : r0 + TT * P, :].rearrange(
                    "(tt p) j -> p tt j", p=P
                ),
                in_=ot.rearrange("p (tt j) -> p tt j", tt=TT),
            )

    for p in reversed((const, ctx_pool, ctxT_pool, asp_pool, aspT_pool, rhsp_pool,
                       atrep_pool, out_pool, psum_t, psum_o, psum_a)):
        p.release()


def kernel(batch_size=None, ctx=None, asp=None, w_u=None, **_unused):
    ctx = np.ascontiguousarray(np.asarray(ctx, dtype=np.float32))
    asp = np.ascontiguousarray(np.asarray(asp, dtype=np.float32))
    w_u = np.ascontiguousarray(np.asarray(w_u, dtype=np.float32))

    nc = build_kernel()
    in_maps = [
        {
            "ctx": ctx[i * PB : (i + 1) * PB],
            "asp": asp[i * PB : (i + 1) * PB],
            "w_u": w_u,
        }
        for i in range(N_CORES)
    ]
    res = run_bass_kernel_spmd(
        nc, in_maps, core_ids=list(range(N_CORES)), **_RUN_KWARGS
    )
    _LAST_RESULTS.clear()
    _LAST_RESULTS.append(res)
    out = np.concatenate(
        [np.asarray(res.results[i]["out"]) for i in range(N_CORES)], axis=0
    )
    return out.astype(np.float32)


# test-harness hooks: extra kwargs for run_bass_kernel_spmd (e.g. trace=True)
# and the last BassKernelResults for profiling. Unused in grading.
_RUN_KWARGS: dict = {}
_LAST_RESULTS: list = []


# revision 9
# speedup vs baseline: 1.7215x; 1.0956x over previous
"""Trainium2 Bass kernel for nn_AlignmentMatrix.

score[b,i,j] = [ctx_i ; asp_j ; ctx_i*asp_j] @ w_u
            = sum_d ctx[b,i,d]*w3[d]*asp[b,j,d] + ctx[b]@w1 + asp[b]@w2

Reformulated per batch as:
    out[b] = ctx[b] @ R[b] + asp_term[b][None, :]
with
    R[b][d, j]  = w3[d]*asp[b,j,d] + w1[d]      (folds the ctx@w1 bias in,
                                                 since sum_d ctx[i,d]*w1[d])
    asp_term[b] = asp[b] @ w2

Data-parallel across the batch dim on 8 NeuronCores (8 batches/core).
Per core, per batch:
  - ctx/asp are DMA'd in with an inline f32->fp16 cast (SWDGE), 4 row-tiles
    per transfer (800KB HBM reads).
  - natural-layout fp16 tiles are transposed on the PE as regular fp16
    matmuls against an identity moving operand (out = dataT @ I) -- this
    gets fast-weight-load + background weight-buffer overlap, unlike
    transpose-mode.  The contraction dim D=400 is covered by four K=128
    blocks [0:128),[128:256),[256:384),[272:400); the overlap of the last
    block is cancelled by zeroing R rows for d in [272,384).
  - R is formed with one fused DVE tensor_scalar (mul w3, add w1) per
    K-block; asp_term is produced already replicated across partitions by
    a w2-replicated stationary matmul.
  - main matmul in fp16 (K=128 everywhere), fp32 PSUM accumulation.
  - DVE adds asp_term during the PSUM->SBUF move, writing fp16 output
    staging; stores are 512KB contiguous DMAs; the host upcasts to f32.
"""

import numpy as np

import concourse.bass as bass
from concourse import bacc
import concourse.mybir as mybir
import concourse.tile as tile
from concourse.bass_utils import run_bass_kernel_spmd
from concourse.masks import make_identity

F32 = mybir.dt.float32
F16 = mybir.dt.float16

B, LC, LA, D = 64, 2048, 512, 400
N_CORES = 8
PB = B // N_CORES  # batches per core
# K blocks of the contraction dim: uniform K=128; the last block overlaps
# [272,384) and is cancelled by zero rows in R.
TD0 = [0, 128, 256, 272]
NCH = 4
PADROWS = 112  # rows of block 3 that must be zero in R (d in [272,384))
P = 128
TT = 4  # row-tiles per DMA transfer


def build_kernel(pb: int = PB, lc: int = LC) -> bass.Bass:
    nc = bacc.Bacc(
        "TRN2",
        target_bir_lowering=False,
        debug=False,
        num_devices=N_CORES,
    )
    ctx_d = nc.dram_tensor("ctx", [pb, lc, D], F32, kind="ExternalInput").ap()
    asp_d = nc.dram_tensor("asp", [pb, LA, D], F32, kind="ExternalInput").ap()
    wu_d = nc.dram_tensor("w_u", [3 * D, 1], F32, kind="ExternalInput").ap()
    out_d = nc.dram_tensor("out", [pb, lc, LA], F16, kind="ExternalOutput").ap()

    with tile.TileContext(nc) as tc:
        _kernel_body(tc, out_d, ctx_d, asp_d, wu_d, pb, lc)
    nc.compile()
    return nc


def _kernel_body(tc, out_d, ctx_d, asp_d, wu_d, pb, lc):
    nc = tc.nc
    n_lc_tiles = lc // P
    n_groups = n_lc_tiles // TT  # ctx tile-groups per batch

    const = tc.alloc_tile_pool(name="const", bufs=1)
    ctx_pool = tc.alloc_tile_pool(name="ctxN", bufs=3)
    ctxT_pool = tc.alloc_tile_pool(name="ctxT", bufs=4)
    asp_pool = tc.alloc_tile_pool(name="aspN", bufs=2)
    aspT_pool = tc.alloc_tile_pool(name="aspT", bufs=2)
    rhsp_pool = tc.alloc_tile_pool(name="rhsp", bufs=2)
    atrep_pool = tc.alloc_tile_pool(name="atrep", bufs=2)
    out_pool = tc.alloc_tile_pool(name="outT", bufs=3)
    psum_t = tc.alloc_tile_pool(name="psumT", bufs=3, space="PSUM")
    psum_o = tc.alloc_tile_pool(name="psumO", bufs=3, space="PSUM")
    psum_a = tc.alloc_tile_pool(name="psumA", bufs=2, space="PSUM")

    ident = const.tile([P, P], F16, tag="ident", name="ident")
    make_identity(nc, ident)

    def pe_transpose(out_ap, in_ap):
        # out = in_ap.T via a regular matmul: stationary = data (FWL-able),
        # moving = identity.  out dtype f32 (PSUM).
        nc.tensor.matmul(out_ap, in_ap, ident, start=True, stop=True)

    # Per K-block weight columns: w1 (add bias), w3 (mul scale) as [*, 1]
    # fp32 per-partition scalars, and w2rep [128, 128] fp16 = w2 replicated
    # along columns (stationary operand producing asp_term replicated over
    # all output partitions).  For block 3 only partitions [112:128) carry
    # data (d in [384,400)); w2rep rows [0:112) are zero.
    w1c, w3c, w2rep = [], [], []
    for c in range(NCH):
        d0 = TD0[c]
        w1t = const.tile([P, 1], F32, tag=f"w1_{c}", name=f"w1_{c}")
        nc.sync.dma_start(out=w1t, in_=wu_d[d0 : d0 + P, :])
        w3t = const.tile([P, 1], F32, tag=f"w3_{c}", name=f"w3_{c}")
        nc.sync.dma_start(out=w3t, in_=wu_d[2 * D + d0 : 2 * D + d0 + P, :])
        w2t = const.tile([P, 1], F32, tag=f"w2_{c}", name=f"w2_{c}")
        nc.sync.dma_start(out=w2t, in_=wu_d[D + d0 : D + d0 + P, :])
        w2r = const.tile([P, P], F16, tag=f"w2rep_{c}", name=f"w2rep_{c}")
        nc.vector.tensor_copy(w2r, w2t.to_broadcast((P, P)))
        if c == 3:
            # Block 3 overlaps blocks 2 on d in [272,384): cancel the
            # double-count by zeroing those rows in every R-side operand.
            nc.vector.memset(w1t[0:PADROWS, :], 0.0)
            nc.vector.memset(w3t[0:PADROWS, :], 0.0)
            nc.vector.memset(w2r[0:PADROWS, :], 0.0)
        w1c.append(w1t)
        w3c.append(w3t)
        w2rep.append(w2r)

    for b in range(pb):
        # ---------------- asp prep for batch b ----------------
        # One 800KB DMA, f32->fp16 cast inline: [512, 400] -> [128, 4*400]
        aspN = asp_pool.tile([P, TT * D], F16, tag="aspN", name=f"aspN_{b}")
        nc.gpsimd.dma_start(
            out=aspN.rearrange("p (tt d) -> p tt d", tt=TT),
            in_=asp_d[b].rearrange("(tt p) d -> p tt d", p=P),
        )

        rhsp = []
        pat = psum_a.tile([P, LA], F32, tag="pat", name=f"pat_{b}")
        for c in range(NCH):
            d0 = TD0[c]
            pT = psum_t.tile([P, TT * P], F32, tag="pT", name=f"pTa_{b}_{c}")
            for t in range(LA // P):
                pe_transpose(
                    pT[:, t * P : (t + 1) * P],
                    aspN[:, t * D + d0 : t * D + d0 + P],
                )
            a_raw = aspT_pool.tile([P, LA], F16, tag=f"aspT_{c}", name=f"aspT_{b}_{c}")
            nc.scalar.copy(a_raw, pT)
            # R block = w3 * aspT + w1 (per-partition fp32 scalars), fp16 out.
            # Block 3's w1/w3 rows [0:112) are zero, so rp rows [0:112) = 0
            # (overlap cancellation) comes out of the same op.
            rp = rhsp_pool.tile([P, LA], F16, tag=f"rhsp_{c}", name=f"rhsp_{b}_{c}")
            nc.vector.tensor_scalar(
                rp,
                a_raw,
                w3c[c],
                w1c[c],
                op0=mybir.AluOpType.mult,
                op1=mybir.AluOpType.add,
            )
            rhsp.append(rp)
            # asp_term replicated: pat[m, j] += sum_d w2[d] * aspT[d, j]
            nc.tensor.matmul(
                pat,
                w2rep[c],
                a_raw,
                start=(c == 0),
                stop=(c == NCH - 1),
            )
        at_rep = atrep_pool.tile([P, LA], F32, tag="atrep", name=f"atrep_{b}")
        nc.scalar.copy(at_rep, pat)

        # ---------------- ctx tiles for batch b ----------------
        for g in range(n_groups):
            # 800KB DMA with inline cast: 4 row-tiles [512, 400] -> [128, 1600]
            ctxN = ctx_pool.tile([P, TT * D], F16, tag="ctxN", name=f"ctxN_{b}_{g}")
            r0 = g * TT * P
            nc.gpsimd.dma_start(
                out=ctxN.rearrange("p (tt d) -> p tt d", tt=TT),
                in_=ctx_d[b, r0 : r0 + TT * P, :].rearrange(
                    "(tt p) d -> p tt d", p=P
                ),
            )
            ot = out_pool.tile([P, TT * LA], F16, tag="ot", name=f"ot_{b}_{g}")
            for tt in range(TT):
                pT = psum_t.tile([P, TT * P], F32, tag="pT", name=f"pTc_{b}_{g}_{tt}")
                for c in range(NCH):
                    d0 = TD0[c]
                    pe_transpose(
                        pT[:, c * P : (c + 1) * P],
                        ctxN[:, tt * D + d0 : tt * D + d0 + P],
                    )
                ctxT = ctxT_pool.tile([P, TT * P], F16, tag="ctxT", name=f"ctxT_{b}_{g}_{tt}")
                nc.scalar.copy(ctxT, pT)

                pO = psum_o.tile([P, LA], F32, tag="pO", name=f"pO_{b}_{g}_{tt}")
                for c in range(NCH):
                    nc.tensor.matmul(
                        pO,
                        ctxT[:, c * P : (c + 1) * P],
                        rhsp[c],
                        start=(c == 0),
                        stop=(c == NCH - 1),
                    )
                nc.vector.tensor_add(ot[:, tt * LA : (tt + 1) * LA], pO, at_rep)
            # contiguous store: [128, 4*512] fp16 -> [512, 512] rows
            nc.sync.dma_start(
                out=out_d[b, r0 : r0 + TT * P, :].rearrange(
                    "(tt p) j -> p tt j", p=P
                ),
                in_=ot.rearrange("p (tt j) -> p tt j", tt=TT),
            )

    for p in reversed((const, ctx_pool, ctxT_pool, asp_pool, aspT_pool, rhsp_pool,
                       atrep_pool, out_pool, psum_t, psum_o, psum_a)):
        p.release()


def kernel(batch_size=None, ctx=None, asp=None, w_u=None, **_unused):
    ctx = np.ascontiguousarray(np.asarray(ctx, dtype=np.float32))
    asp = np.ascontiguousarray(np.asarray(asp, dtype=np.float32))
    w_u = np.ascontiguousarray(np.asarray(w_u, dtype=np.float32))

    nc = build_kernel()
    in_maps = [
        {
            "ctx": ctx[i * PB : (i + 1) * PB],
            "asp": asp[i * PB : (i + 1) * PB],
            "w_u": w_u,
        }
        for i in range(N_CORES)
    ]
    res = run_bass_kernel_spmd(
        nc, in_maps, core_ids=list(range(N_CORES)), **_RUN_KWARGS
    )
    _LAST_RESULTS.clear()
    _LAST_RESULTS.append(res)
    out = np.concatenate(
        [np.asarray(res.results[i]["out"]) for i in range(N_CORES)], axis=0
    )
    return out.astype(np.float32)


# test-harness hooks: extra kwargs for run_bass_kernel_spmd (e.g. trace=True)
# and the last BassKernelResults for profiling. Unused in grading.
_RUN_KWARGS: dict = {}
_LAST_RESULTS: list = []
